# revision 2
# baseline (speedup 1.0000x reference)
"""GateAttention (GAU squared-relu causal attention) Trainium2 Bass kernel.

Problem: B=8, L=2048, E=128, DV=1024
  scores = q @ k^T / sqrt(E)            [B, L, L], causal mask
  A      = relu(scores)^2 / (m+1)       (m+1 = # valid keys in row m)
  out    = u * (A @ v)

Sharding: data-parallel over batch — core b computes batch b (SPMD, no
collectives). Causality is exploited analytically (the attn_mask input is
a deterministic triangular causal mask), halving compute and skipping the
33MB mask load entirely.

Per-core dataflow (one batch: q,k [2048,128], u,v,out [2048,1024]):
  1. PE-transpose q,k into resident qT,kT [128E, 2048L] (E on partitions).
  2. v fully resident in SBUF (8MB).
  3. m-outer loop in groups of 512 queries:
     stage 1: scoresT chunk [128n, 512m] = kT_tile^T @ qT_slice (fp32r
              matmul -> full PE rate), ACT relu PSUM->SBUF, DVE square
              (+ causal 0/1 mask multiply on diagonal chunks).
     stage 2: per m_tile accumulate A^T^T @ v over all n_tiles <= m in a
              PSUM [128,1024] tile; finalize = ACT copy with per-partition
              scale 1/(E*(m+1)), DVE multiply by u, DMA out.
"""

import numpy as np

import concourse.bacc as bacc
import concourse.mybir as mybir
import concourse.tile as tile
from concourse.bass_utils import run_bass_kernel_spmd

B, L, E, DV = 8, 2048, 1024 // 8, 1024
P = 128                      # partitions
MT = L // P                  # 16 m tiles of 128 queries
NT = L // P                  # 16 n tiles of 128 keys
G = 4                        # m tiles per group
NG = MT // G                 # 4 groups
MG = P * G                   # 512 queries per group

F32 = mybir.dt.float32
F32R = mybir.dt.float32r
AFT = mybir.ActivationFunctionType

# consts tensor column layout
C_MASKS = 0                  # 4 masks [128, 512] at cols 512*j
C_IDENT = 4 * 512            # identity [128, 128]
C_RSCALE = C_IDENT + P       # rowscale [128, 16]
C_COLS = C_RSCALE + MT


def make_consts() -> np.ndarray:
    c = np.zeros((P, C_COLS), dtype=np.float32)
    # causal keep-masks for diagonal chunks: for n_tile j within a group,
    # keep scoresT[n_local, m_local] iff m_local >= n_local + 128*j
    f = np.arange(512)[None, :]
    p = np.arange(P)[:, None]
    for j in range(4):
        c[:, 512 * j:512 * (j + 1)] = (f >= p + P * j).astype(np.float32)
    c[:, C_IDENT:C_IDENT + P] = np.eye(P, dtype=np.float32)
    # rowscale[p, t] = 1 / (E * (m+1)) with m = 128*t + p
    t = np.arange(MT)[None, :]
    c[:, C_RSCALE:C_RSCALE + MT] = 1.0 / (E * (P * t + p + 1.0))
    return c


def build_kernel(nc, tc, q_d, k_d, v_d, u_d, c_d, o_d):
    with (
        tc.tile_pool(name="const", bufs=1) as cpool,
        tc.tile_pool(name="qkt", bufs=1) as qkt_pool,
        tc.tile_pool(name="vres", bufs=1) as v_pool,
        tc.tile_pool(name="stage", bufs=3) as stg,
        tc.tile_pool(name="at", bufs=24) as at_pool,
        tc.tile_pool(name="work", bufs=4) as wk,
        tc.tile_pool(name="uo", bufs=2) as uo_pool,
        tc.tile_pool(name="ps_s", bufs=3, space="PSUM") as ps_s,
        tc.tile_pool(name="ps_o", bufs=2, space="PSUM") as ps_o,
    ):
        consts = cpool.tile([P, C_COLS], F32)
        nc.sync.dma_start(out=consts, in_=c_d)
        masks = [consts[:, 512 * j:512 * (j + 1)] for j in range(4)]
        ident = consts[:, C_IDENT:C_IDENT + P]

        qT = qkt_pool.tile([P, L], F32R, tag="qT")
        kT = qkt_pool.tile([P, L], F32R, tag="kT")

        # ---- load q, k and transpose into qT, kT ----
        for src, dst in ((q_d, qT), (k_d, kT)):
            for c in range(L // 512):
                st = stg.tile([P, 4, P], F32, tag="stg")
                nc.sync.dma_start(
                    out=st,
                    in_=src[512 * c:512 * (c + 1), :].rearrange(
                        "(t p) e -> p t e", p=P),
                )
                for t in range(4):
                    ps = ps_s.tile([P, 512], F32, tag="ps_s")
                    nc.tensor.transpose(ps[:, 0:P], st[:, t, :], ident)
                    mt = 4 * c + t
                    nc.scalar.copy(out=dst[:, P * mt:P * (mt + 1)],
                                   in_=ps[:, 0:P])

        # ---- resident v ----
        v_sb = []
        for n in range(NT):
            vt = v_pool.tile([P, DV], F32R, tag=f"v{n}")
            nc.sync.dma_start(out=vt, in_=v_d[P * n:P * (n + 1), :])
            v_sb.append(vt)

        # ---- main loop over m groups ----
        for g in range(NG):
            m0 = MG * g
            n_hi = G * (g + 1)
            at_tiles = []
            for n in range(n_hi):
                ps = ps_s.tile([P, MG], F32, tag="ps_s")
                nc.tensor.matmul(
                    ps,
                    kT[:, P * n:P * (n + 1)],
                    qT[:, m0:m0 + MG],
                    start=True, stop=True,
                )
                r = wk.tile([P, MG], F32, tag="r")
                nc.scalar.activation(r, ps, AFT.Relu)
                at = at_pool.tile([P, MG], F32R, tag="at")
                if n >= G * g:  # diagonal chunk: causal mask
                    rm = wk.tile([P, MG], F32, tag="rm")
                    nc.vector.tensor_mul(rm, r, masks[n - G * g])
                    nc.vector.tensor_mul(at, r, rm)
                else:
                    nc.vector.tensor_mul(at, r, r)
                at_tiles.append(at)

            for j in range(G):
                mt = G * g + j
                po = ps_o.tile([P, DV], F32, tag="ps_o")
                for n in range(mt + 1):
                    lhsT = at_tiles[n][:, P * j:P * (j + 1)]
                    for h in range(2):
                        nc.tensor.matmul(
                            po[:, 512 * h:512 * (h + 1)],
                            lhsT,
                            v_sb[n][:, 512 * h:512 * (h + 1)],
                            start=(n == 0), stop=(n == mt),
                        )
                ut = uo_pool.tile([P, DV], F32, tag="u")
                nc.sync.dma_start(out=ut, in_=u_d[P * mt:P * (mt + 1), :])
                tmp = uo_pool.tile([P, DV], F32, tag="tmp")
                nc.scalar.activation(
                    tmp, po, AFT.Copy,
                    scale=consts[:, C_RSCALE + mt:C_RSCALE + mt + 1])
                ot = uo_pool.tile([P, DV], F32, tag="ot")
                nc.vector.tensor_mul(ot, tmp, ut)
                nc.sync.dma_start(out=o_d[P * mt:P * (mt + 1), :], in_=ot)


def build_program():
    nc = bacc.Bacc("TRN2", target_bir_lowering=False, debug=False,
                   num_devices=B)
    q_d = nc.dram_tensor("q", [L, E], F32, kind="ExternalInput").ap()
    k_d = nc.dram_tensor("k", [L, E], F32, kind="ExternalInput").ap()
    v_d = nc.dram_tensor("v", [L, DV], F32R, kind="ExternalInput").ap()
    u_d = nc.dram_tensor("u", [L, DV], F32, kind="ExternalInput").ap()
    c_d = nc.dram_tensor("consts", [P, C_COLS], F32,
                         kind="ExternalInput").ap()
    o_d = nc.dram_tensor("out", [L, DV], F32, kind="ExternalOutput").ap()

    with tile.TileContext(nc) as tc:
        build_kernel(nc, tc, q_d, k_d, v_d, u_d, c_d, o_d)
    nc.compile()
    return nc


_NC_CACHE = None


def kernel(u, q, k, v, attn_mask=None, trace=False):
    """Full inputs in, full output out. attn_mask ignored (deterministic
    causal)."""
    global _NC_CACHE
    if _NC_CACHE is None:
        _NC_CACHE = build_program()
    nc = _NC_CACHE

    consts = make_consts()
    in_maps = [
        {
            "q": np.ascontiguousarray(q[b], dtype=np.float32),
            "k": np.ascontiguousarray(k[b], dtype=np.float32),
            "v": np.ascontiguousarray(v[b], dtype=np.float32),
            "u": np.ascontiguousarray(u[b], dtype=np.float32),
            "consts": consts,
        }
        for b in range(B)
    ]
    res = run_bass_kernel_spmd(nc, in_maps, list(range(B)), trace=trace)
    out = np.stack([res.results[b]["out"] for b in range(B)])
    if trace:
        kernel.last_results = res
    return out


# revision 12
# speedup vs baseline: 1.0803x; 1.0803x over previous
"""GateAttention (GAU squared-relu causal attention) Trainium2 Bass kernel.

Problem: B=8, L=2048, E=128, DV=1024
  scores = q @ k^T / sqrt(E)            [B, L, L], causal mask
  A      = relu(scores)^2 / (m+1)       (m+1 = # valid keys in row m)
  out    = u * (A @ v)

Sharding: data-parallel over batch — core b computes batch b (SPMD, no
collectives). Causality is exploited analytically (the attn_mask input is
a deterministic triangular causal mask), halving compute and skipping the
33MB mask load entirely.

Per-core dataflow (one batch: q,k [2048,128], u,v,out [2048,1024]):
  1. PE-transpose q,k into resident qT,kT [128E, 2048L] (E on partitions).
  2. v fully resident in SBUF (8MB, fp32r).
  3. m-outer loop in groups of 512 queries (software-pipelined one group
     ahead on stage 1):
     stage 1: scoresT chunk [128n, m...] = kT_tile^T @ qT_slice (fp32r
              matmul -> full PE rate; diagonal chunks start at the
              diagonal), ACT relu PSUM->SBUF, DVE square into f32r A^T
              (+ one [128,128] triangular mask multiply on the exact
              diagonal block).
     stage 2: per m_tile accumulate A^T^T @ v over all n_tiles <= m in a
              PSUM [128,1024] tile; finalize = ACT copy with per-partition
              scale 1/(E*(m+1)), DVE multiply by u, DMA out.
"""

import numpy as np

import concourse.bacc as bacc
import concourse.mybir as mybir
import concourse.tile as tile
from concourse.bass_utils import run_bass_kernel_spmd

B, L, E, DV = 8, 2048, 1024 // 8, 1024
P = 128                      # partitions
MT = L // P                  # 16 m tiles of 128 queries
NT = L // P                  # 16 n tiles of 128 keys
G = 4                        # m tiles per group
NG = MT // G                 # 4 groups
MG = P * G                   # 512 queries per group

F32 = mybir.dt.float32
F32R = mybir.dt.float32r
AFT = mybir.ActivationFunctionType

# consts tensor column layout
C_IDENT = 0                  # identity [128, 128] (first: unblocks PE)
C_TRI = P                    # lower-tri keep mask [128, 128]
C_RSCALE = C_TRI + P         # rowscale [128, 16]
C_COLS = C_RSCALE + MT


def make_consts() -> np.ndarray:
    c = np.zeros((P, C_COLS), dtype=np.float32)
    c[:, C_IDENT:C_IDENT + P] = np.eye(P, dtype=np.float32)
    # diagonal-block causal keep mask: keep iff m_local >= n_local
    f = np.arange(P)[None, :]
    p = np.arange(P)[:, None]
    c[:, C_TRI:C_TRI + P] = (f >= p).astype(np.float32)
    # rowscale[p, t] = 1 / (E * (m+1)) with m = 128*t + p
    t = np.arange(MT)[None, :]
    c[:, C_RSCALE:C_RSCALE + MT] = 1.0 / (E * (P * t + p + 1.0))
    return c


def build_kernel(nc, tc, q_d, k_d, v_d, u_d, c_d, o_d):
    with (
        tc.tile_pool(name="const", bufs=1) as cpool,
        tc.tile_pool(name="qkt", bufs=1) as qkt_pool,
        tc.tile_pool(name="vres", bufs=1) as v_pool,
        tc.tile_pool(name="stage", bufs=3) as stg,
        tc.tile_pool(name="at", bufs=32) as at_pool,
        tc.tile_pool(name="work", bufs=2) as wk,
        tc.tile_pool(name="upool", bufs=6) as u_pool,
        tc.tile_pool(name="uo", bufs=4) as uo_pool,
        tc.tile_pool(name="ps_s", bufs=4, space="PSUM") as ps_s,
        tc.tile_pool(name="ps_o", bufs=4, space="PSUM") as ps_o,
    ):
        consts = cpool.tile([P, C_COLS], F32)
        # identity first (tiny) so PE transposes can start immediately;
        # the rest (tri mask, rowscale) is loaded after the staging DMAs
        nc.sync.dma_start(out=consts[:, 0:C_TRI], in_=c_d[:, 0:C_TRI])
        ident = consts[:, C_IDENT:C_IDENT + P]
        tri = consts[:, C_TRI:C_TRI + P]

        qT = qkt_pool.tile([P, L], F32R, tag="qT")
        kT = qkt_pool.tile([P, L], F32R, tag="kT")

        # ---- load q, k and transpose into qT, kT ----
        def transpose_chunk(src, dst, c):
            st = stg.tile([P, 4, P], F32, tag="stg")
            nc.sync.dma_start(
                out=st,
                in_=src[512 * c:512 * (c + 1), :].rearrange(
                    "(t p) e -> p t e", p=P),
            )
            for t in range(4):
                ps = ps_s.tile([P, MG], F32, tag="ps_s")
                nc.tensor.transpose(ps[:, 0:P], st[:, t, :], ident)
                mt = 4 * c + t
                nc.scalar.copy(out=dst[:, P * mt:P * (mt + 1)],
                               in_=ps[:, 0:P])

        v_tiles = [None] * NT

        def load_v(n):
            vt = v_pool.tile([P, DV], F32R, tag=f"v{n}")
            nc.sync.dma_start(out=vt, in_=v_d[P * n:P * (n + 1), :])
            v_tiles[n] = vt

        # ---- stage 1 for one m-group: produce A^T tiles ----
        def stage1(g):
            m0 = MG * g
            tiles = []
            for n in range(G * (g + 1)):
                jj = n - G * g        # >=0 on diagonal chunks
                off = max(jj, 0) * P  # start at the diagonal
                w = MG - off
                ps = ps_s.tile([P, MG], F32, tag="ps_s")
                nc.tensor.matmul(
                    ps[:, 0:w],
                    kT[:, P * n:P * (n + 1)],
                    qT[:, m0 + off:m0 + MG],
                    start=True, stop=True,
                )
                r = wk.tile([P, MG], F32, tag="r")
                nc.scalar.activation(r[:, 0:w], ps[:, 0:w], AFT.Relu)
                at = at_pool.tile([P, MG], F32R, tag="at")
                if jj >= 0:
                    # exact diagonal block: triangular mask, in place
                    nc.vector.tensor_mul(r[:, 0:P], r[:, 0:P], tri)
                # square into f32r A^T at column offset `off`
                nc.vector.tensor_mul(at[:, off:MG], r[:, 0:w], r[:, 0:w])
                tiles.append(at)
            return tiles

        # ---- stage 2 for one m_tile ----
        def stage2_mtile(g, j, at_tiles):
            mt = G * g + j
            ut = u_tiles[mt]
            rs = consts[:, C_RSCALE + mt:C_RSCALE + mt + 1]
            # two independent PSUM half-tiles; h-inner accumulation
            po = [ps_o.tile([P, 512], F32, tag="ps_o", name=f"po{mt}_{hh}")
                  for hh in range(2)]
            for n in range(mt + 1):
                at = at_tiles[n]
                for h in range(2):
                    nc.tensor.matmul(
                        po[h],
                        at[:, P * j:P * (j + 1)],
                        v_tiles[n][:, 512 * h:512 * (h + 1)],
                        start=(n == 0), stop=(n == mt),
                    )
            ot = uo_pool.tile([P, DV], F32, tag="ot")
            for h in range(2):
                lo, hi = 512 * h, 512 * (h + 1)
                nc.scalar.activation(ot[:, lo:hi], po[h],
                                     AFT.Copy, scale=rs)
                nc.vector.tensor_mul(ot[:, lo:hi], ot[:, lo:hi],
                                     ut[:, lo:hi])
                # store issued from ACT: by the time ACT reaches this
                # trigger the DVE mul is already done -> no stream stall
                nc.scalar.dma_start(out=o_d[P * mt:P * (mt + 1), lo:hi],
                                    in_=ot[:, lo:hi])

        u_tiles = [None] * MT

        def load_u(mt):
            ut = u_pool.tile([P, DV], F32, tag="u")
            nc.sync.dma_start(out=ut, in_=u_d[P * mt:P * (mt + 1), :])
            u_tiles[mt] = ut

        # ---- emission order: staging/transposes first, then v/u loads
        # interleaved in need order on the single sync queue ----
        transpose_chunk(q_d, qT, 0)
        transpose_chunk(k_d, kT, 0)
        nc.sync.dma_start(out=consts[:, C_TRI:], in_=c_d[:, C_TRI:])
        at_cur = stage1(0)
        for c in range(1, L // 512):
            transpose_chunk(q_d, qT, c)
            transpose_chunk(k_d, kT, c)
        for n in range(4):
            load_v(n)
        # fine-grained interleave: v_n arrives just before stage2 needs it,
        # u_mt just before its finalize
        order = []
        vn, un = 4, 0
        while vn < NT or un < MT:
            for _ in range(2):
                if un < MT and (un * 2 <= vn or vn >= NT):
                    order.append(("u", un)); un += 1
            for _ in range(2):
                if vn < NT:
                    order.append(("v", vn)); vn += 1
        for kind, idx in order:
            (load_u if kind == "u" else load_v)(idx)

        # ---- main loop, stage 1 pipelined one group ahead ----
        for g in range(NG):
            stage2_mtile(g, 0, at_cur)
            at_next = stage1(g + 1) if g + 1 < NG else None
            for j in range(1, G):
                stage2_mtile(g, j, at_cur)
            at_cur = at_next


def build_program():
    nc = bacc.Bacc("TRN2", target_bir_lowering=False, debug=False,
                   num_devices=B)
    q_d = nc.dram_tensor("q", [L, E], F32, kind="ExternalInput").ap()
    k_d = nc.dram_tensor("k", [L, E], F32, kind="ExternalInput").ap()
    v_d = nc.dram_tensor("v", [L, DV], F32R, kind="ExternalInput").ap()
    u_d = nc.dram_tensor("u", [L, DV], F32, kind="ExternalInput").ap()
    c_d = nc.dram_tensor("consts", [P, C_COLS], F32,
                         kind="ExternalInput").ap()
    o_d = nc.dram_tensor("out", [L, DV], F32, kind="ExternalOutput").ap()

    with tile.TileContext(nc) as tc:
        build_kernel(nc, tc, q_d, k_d, v_d, u_d, c_d, o_d)
    nc.compile()
    return nc


_NC_CACHE = None


def kernel(u, q, k, v, attn_mask=None, trace=False):
    """Full inputs in, full output out. attn_mask ignored (deterministic
    causal)."""
    global _NC_CACHE
    if _NC_CACHE is None:
        _NC_CACHE = build_program()
    nc = _NC_CACHE

    consts = make_consts()
    in_maps = [
        {
            "q": np.ascontiguousarray(q[b], dtype=np.float32),
            "k": np.ascontiguousarray(k[b], dtype=np.float32),
            "v": np.ascontiguousarray(v[b], dtype=np.float32),
            "u": np.ascontiguousarray(u[b], dtype=np.float32),
            "consts": consts,
        }
        for b in range(B)
    ]
    res = run_bass_kernel_spmd(nc, in_maps, list(range(B)), trace=trace)
    out = np.stack([res.results[b]["out"] for b in range(B)])
    if trace:
        kernel.last_results = res
    return out
